# revision 8
# baseline (speedup 1.0000x reference)
"""Multi-head dot-product attention (B=2, Q=K=2048, EMB=2048, H=32, D=64) on 8 TRN2 cores.

Sharding: data parallel over batch (2) x tensor parallel over heads (4 groups of 8).
Core c handles batch c//4, heads 8*(c%4) .. 8*(c%4)+8. Each core computes a partial
output [2048, 2048] (its heads' contribution through wo) in bf16; host sums the 4
head-group partials per batch in f32.

v5: the scalar engine's exp stream (~294us over 256 [128,1024] tiles) is the
attention pacer; the PE (~437us of bf16 matmul streaming) must deliver score
tiles at the scalar's rate *steadily* while soaking its own slack with finely-
grained independent work. Structure:
  - attention processes one head per (pair, hh) pass: per kc the PE owes only
    0.85us (scores+ctx) against the scalar's 1.15us, and the scores pipeline
    runs 2 kc ahead (3 psum slots) with ctx lagging 1 kc, so the
    scores->exp->mul->ctx round trip never blocks the in-order PE queue
  - independent PE work (q-projections for later chunks, out-projection of
    finished rows) is emitted in <=1.7us slices, 4 slots per pass, so the
    scalar never starves behind a long burst
  - K and V projections fill the pipe up front (scores consume all of kT/v
    within the first pass); exp(bias^T) is precomputed on the host
"""

import numpy as np
import ml_dtypes
from contextlib import ExitStack

import concourse.bass as bass
from concourse import bacc
import concourse.mybir as mybir
import concourse.tile as tile
from concourse.bass_utils import run_bass_kernel_spmd

BF16 = mybir.dt.bfloat16
F32 = mybir.dt.float32
AF = mybir.ActivationFunctionType

B, T, E = 2, 2048, 2048          # batch, tokens (Q=K), embed
H, D = 32, 64                     # total heads, head dim
NH = 8                            # heads per core
HD = NH * D                       # 512, per-core head-dim total
EC = E // 128                     # 16 contraction chunks
KC = T // 128                     # 16 key chunks
QCH = 1024                        # attention q-chunk (psum + exp tile width)
NQC = T // QCH                    # 2
N_CORES = 8


def build_program():
    nc = bacc.Bacc("TRN2", target_bir_lowering=False, debug=False,
                   num_devices=N_CORES)

    xqT = nc.dram_tensor("xqT", [E, T], BF16, kind="ExternalInput").ap()
    xkvT = nc.dram_tensor("xkvT", [E, T], BF16, kind="ExternalInput").ap()
    expbT = nc.dram_tensor("expbT", [T, T], BF16, kind="ExternalInput").ap()
    wq = nc.dram_tensor("wq", [E, HD], BF16, kind="ExternalInput").ap()
    wk = nc.dram_tensor("wk", [E, HD], BF16, kind="ExternalInput").ap()
    wv = nc.dram_tensor("wv", [E, HD], BF16, kind="ExternalInput").ap()
    wo = nc.dram_tensor("wo", [HD, E], BF16, kind="ExternalInput").ap()
    out = nc.dram_tensor("out", [T, E], BF16, kind="ExternalOutput").ap()

    with tile.TileContext(nc) as tc, ExitStack() as ctx:
        persist = ctx.enter_context(tc.tile_pool(name="persist", bufs=1))
        qT_sb = persist.tile([128, HD // 128, T], BF16)   # q^T[hd, t]
        kT_sb = persist.tile([128, HD // 128, T], BF16)
        v_sb = persist.tile([128, KC, NH, D + 1], BF16)   # v[k, h, d] + ones
        ctxT_sb = persist.tile([128, HD // 128, T], BF16)
        nc.vector.memset(v_sb[:, :, :, D:D + 1], 1.0)

        # wk's slot is reused for wo later (same tag)
        wpool = ctx.enter_context(tc.tile_pool(name="wpool", bufs=1))
        wk_sb = wpool.tile([128, EC, HD], BF16, tag="wkwo", name="wk_sb")
        wq_sb = wpool.tile([128, EC, HD], BF16, tag="wq", name="wq_sb")
        wv_sb = wpool.tile([128, EC, HD], BF16, tag="wv", name="wv_sb")

        expbp = ctx.enter_context(tc.tile_pool(name="expbp", bufs=1))
        xsp = ctx.enter_context(tc.tile_pool(name="xsp", bufs=2))
        # psum: 3 x s[128,1024] (6 banks) + 1 x ctx[65,1024] (2 banks)
        psp = ctx.enter_context(tc.tile_pool(name="psp", bufs=3, space="PSUM"))
        atp = ctx.enter_context(tc.tile_pool(name="atp", bufs=3))
        a2p = ctx.enter_context(tc.tile_pool(name="a2p", bufs=4))
        nrm = ctx.enter_context(tc.tile_pool(name="nrm", bufs=2))
        nrmd = ctx.enter_context(tc.tile_pool(name="nrmd", bufs=2, space="DRAM"))

        # ---------------- emission helpers ----------------
        def dma_chunked(dst, src_re, nchunk):
            step = EC // nchunk
            for i in range(nchunk):
                nc.gpsimd.dma_start(out=dst[:, i * step:(i + 1) * step, :],
                                    in_=src_re[:, i * step:(i + 1) * step, :])

        def x_stream(src, tc4, nchunk=1):
            """Stream activation chunk [128, EC, 512] for token cols tc4*512.."""
            xs = xsp.tile([128, EC, 512], BF16, name="xs", tag="xs")
            step = EC // nchunk
            for i in range(nchunk):
                nc.sync.dma_start(
                    out=xs[:, i * step:(i + 1) * step, :],
                    in_=bass.AP(tensor=src.tensor,
                                offset=src.offset + tc4 * 512 + i * step * 128 * T,
                                ap=[[T, 128], [128 * T, step], [1, 512]]))
            return xs

        def proj_mms(ps, w_sb, x_sb, hdc, e0, e1):
            for ec in range(e0, e1):
                nc.tensor.matmul(ps[:, 0:512],
                                 lhsT=w_sb[:, ec, hdc * 128:(hdc + 1) * 128],
                                 rhs=x_sb[:, ec, :],
                                 start=(ec == 0), stop=(ec == EC - 1))

        def proj_tile(w_sb, x_sb, dst, hdc, tc4):
            ps = psp.tile([128, QCH], F32, tag="s", name="ps")
            proj_mms(ps, w_sb, x_sb, hdc, 0, EC)
            nc.vector.tensor_copy(dst[:, hdc, tc4 * 512:(tc4 + 1) * 512],
                                  ps[:, 0:512])

        def v_tile(x_sb, tc4, sub):
            kc = tc4 * 4 + sub
            ps = psp.tile([128, QCH], F32, tag="s", name="vps")
            for ec in range(EC):
                nc.tensor.matmul(ps[:, 0:512],
                                 lhsT=x_sb[:, ec, sub * 128:(sub + 1) * 128],
                                 rhs=wv_sb[:, ec, :],
                                 start=(ec == 0), stop=(ec == EC - 1))
            nc.vector.tensor_copy(
                v_sb[:, kc, :, 0:D],
                ps[:, 0:512].rearrange("p (h d) -> p h d", h=NH))

        def expb_tile(kc, qc):
            eb = expbp.tile([128, QCH], BF16, tag=f"e{kc}", name=f"eb{kc}")
            nc.gpsimd.dma_start(
                out=eb[:],
                in_=expbT[kc * 128:(kc + 1) * 128, qc * QCH:(qc + 1) * QCH])
            return eb

        wo_ref = [None]

        def out_tile(i, copy_eng="v"):
            """Out-projection psum tile i (of 64): tc16 = i//4, e-quarter = i%4."""
            tc16, eq = i // 4, i % 4
            po = psp.tile([128, 512], F32, tag="s", name="po")
            for hdc in range(HD // 128):
                nc.tensor.matmul(
                    po[:],
                    lhsT=ctxT_sb[:, hdc, tc16 * 128:(tc16 + 1) * 128],
                    rhs=wo_ref[0][:, hdc, eq * 512:(eq + 1) * 512],
                    start=(hdc == 0), stop=(hdc == HD // 128 - 1))
            ot = nrm.tile([128, 512], BF16, tag="ostage", name="ot")
            if copy_eng == "v":
                nc.vector.tensor_copy(ot[:], po[:])
            else:
                nc.scalar.activation(ot[:], po[:], AF.Copy)
            nc.sync.dma_start(
                out=out[tc16 * 128:(tc16 + 1) * 128, eq * 512:(eq + 1) * 512],
                in_=ot[:])

        def emit_score(pair, hh, qc, kc):
            pr = slice(hh * D, (hh + 1) * D)
            s = psp.tile([128, QCH], F32, tag="s", name="s")
            for half in range(QCH // 512):
                q0 = qc * QCH + half * 512
                nc.tensor.matmul(
                    s[:, half * 512:(half + 1) * 512],
                    lhsT=kT_sb[pr, pair, kc * 128:(kc + 1) * 128],
                    rhs=qT_sb[pr, pair, q0:q0 + 512],
                    start=True, stop=True)
            return s

        def emit_expmul(s_tile, eb):
            at = atp.tile([128, QCH], BF16, tag="at", name="at")
            nc.scalar.activation(at[:], s_tile[:], AF.Exp)
            a2 = a2p.tile([128, QCH], BF16, tag="a2", name="a2")
            nc.vector.tensor_mul(a2[:], at[:], eb[:])
            return a2

        def emit_ctx(ctx_t, pair, hh, kc, a2):
            h = pair * 2 + hh
            for half in range(QCH // 512):
                nc.tensor.matmul(
                    ctx_t[:, half * 512:(half + 1) * 512],
                    lhsT=v_sb[:, kc, h, :],
                    rhs=a2[:, half * 512:(half + 1) * 512],
                    start=(kc == 0), stop=(kc == KC - 1))

        def norm_hh(ctx_t, pair, hh, qc):
            ctxf = nrm.tile([D + 1, QCH], BF16, tag="ctxf", name="ctxf")
            nc.vector.tensor_copy(ctxf[:], ctx_t[:])  # frees the psum slot
            srow = nrm.tile([128, QCH // 128], F32, tag="srow", name="srow")
            nc.gpsimd.dma_start(out=srow[:], in_=ctxf[D:D + 1, :])
            rec = nrm.tile([128, QCH // 128], F32, tag="rec", name="rec")
            nc.vector.reciprocal_approx_fast(out=rec[:], in_=srow[:])
            rec_d = nrmd.tile([QCH], F32, tag="recd", name="recd")
            nc.sync.dma_start(
                out=rec_d[:].rearrange("(p j) -> p j", p=128), in_=rec[:])
            recb = nrm.tile([D, QCH], BF16, tag="recb", name="recb")
            rd = rec_d[:]
            bcast = bass.AP(tensor=rd.tensor, offset=rd.offset,
                            ap=[[0, D]] + list(rd.ap))
            nc.gpsimd.dma_start(out=recb[:], in_=bcast)  # casts f32->bf16
            if hh == 0:
                nc.vector.tensor_mul(
                    ctxT_sb[0:D, pair, qc * QCH:(qc + 1) * QCH],
                    ctxf[0:D, :], recb[:])
            else:
                stage = nrm.tile([D, QCH], BF16, tag="cstage", name="stg")
                nc.vector.tensor_mul(stage[:], ctxf[0:D, :], recb[:])
                nc.sync.dma_start(
                    out=ctxT_sb[D:2 * D, pair, qc * QCH:(qc + 1) * QCH],
                    in_=stage[:])

        # ---------------- fill phase ----------------
        dma_chunked(wk_sb, wk.rearrange("(ec p) n -> p ec n", p=128), 4)
        nc.gpsimd.dma_start(out=wv_sb[:],
                            in_=wv.rearrange("(ec p) n -> p ec n", p=128))
        dma_chunked(wq_sb, wq.rearrange("(ec p) n -> p ec n", p=128), 2)
        expb_tiles = {kc: expb_tile(kc, 0) for kc in range(KC)}

        xq_fill = []
        for tc4 in range(4):
            xs = x_stream(xkvT, tc4, nchunk=(4 if tc4 == 0 else 1))
            for hdc in range(4):
                proj_tile(wk_sb, xs, kT_sb, hdc, tc4)
            for sub in range(4):
                v_tile(xs, tc4, sub)
        for tc4 in range(2):
            xs = x_stream(xqT, tc4)
            xq_fill.append(xs)
        # q-projection for (pair0, qc0) only; the rest interleaves into attention
        proj_tile(wq_sb, xq_fill[0], qT_sb, 0, 0)
        proj_tile(wq_sb, xq_fill[1], qT_sb, 0, 1)

        # wo reuses wk's slot; its DMA waits on the last wk reader
        wo_sb = wpool.tile([128, HD // 128, E], BF16, tag="wkwo", name="wo_sb")
        nc.gpsimd.dma_start(out=wo_sb[:],
                            in_=wo.rearrange("(c p) n -> p c n", p=128))
        wo_ref[0] = wo_sb

        # ------------- interleave supply (fine-grained thunks) -------------
        xq_late = {}

        def make_proj_halves(src_tiles, hdc, tc4):
            """A q-projection tile as two 8-matmul thunks."""
            state = {}

            def first():
                state["ps"] = psp.tile([128, QCH], F32, tag="s", name="ps")
                proj_mms(state["ps"], wq_sb, src_tiles[tc4], hdc, 0, EC // 2)

            def second():
                proj_mms(state["ps"], wq_sb, src_tiles[tc4], hdc, EC // 2, EC)
                nc.vector.tensor_copy(
                    qT_sb[:, hdc, tc4 * 512:(tc4 + 1) * 512],
                    state["ps"][:, 0:512])
            return [first, second]

        def make_stream_thunk(tc4):
            def go():
                xq_late[tc4] = x_stream(xqT, tc4)
            return [go]

        supply = {0: [], 1: []}
        xq_fill_d = {0: xq_fill[0], 1: xq_fill[1]}
        for p in (1, 2, 3):                      # q-proj qc0 for pairs 1-3
            for tc4 in (0, 1):
                supply[0] += make_proj_halves(xq_fill_d, p, tc4)
        supply[0] += make_stream_thunk(2) + make_stream_thunk(3)
        for p in (0, 1, 2, 3):                   # q-proj qc1
            for tc4 in (2, 3):
                supply[0] += make_proj_halves(xq_late, p, tc4)
        supply[1] += [lambda i=i: out_tile(i) for i in range(32)]

        def slot_budget(qc, win, kc):
            """Interleave thunks to emit at (window, kc). Tuned so supA
            (q-proj qc0 + xq streams, 14 thunks) drains in windows 0-1 and
            supB (q-proj qc1, needs the late xq streams landed) starts only
            from window 2's tail."""
            if qc == 1:
                return 1 if kc % 4 == 2 else 0
            if win <= 1:
                return 2 if kc % 4 == 2 else 0
            if win == 2:
                return 1 if kc in (10, 14) else 0
            return 1 if kc % 4 == 2 else 0

        # ---------------- attention ----------------
        for qc in range(NQC):
            sup = supply[qc]
            si = 0
            for pair in range(NH // 2):
                for hh in range(2):
                    win = pair * 2 + hh
                    ctx_t = psp.tile([D + 1, QCH], F32, tag="ctx", bufs=1,
                                     name="ctx")
                    s_tiles = {0: emit_score(pair, hh, qc, 0),
                               1: emit_score(pair, hh, qc, 1)}
                    a2_hist = {}
                    for kc in range(KC):
                        a2_hist[kc] = emit_expmul(s_tiles.pop(kc),
                                                  expb_tiles[kc])
                        if qc == 0 and pair == 3 and hh == 1:
                            expb_tiles[kc] = expb_tile(kc, 1)
                        if kc + 2 < KC:
                            s_tiles[kc + 2] = emit_score(pair, hh, qc, kc + 2)
                        for _ in range(slot_budget(qc, win, kc)):
                            if si < len(sup):
                                sup[si]()
                                si += 1
                        if kc >= 1:
                            emit_ctx(ctx_t, pair, hh, kc - 1,
                                     a2_hist.pop(kc - 1))
                    emit_ctx(ctx_t, pair, hh, KC - 1, a2_hist.pop(KC - 1))
                    norm_hh(ctx_t, pair, hh, qc)
            # drain any unused supply at qc end
            while si < len(sup):
                sup[si]()
                si += 1

        # ---------------- tail: out-proj for qc1 rows ----------------
        for i in range(32, 64):
            out_tile(i, copy_eng=("s" if i % 2 else "v"))

    nc.compile()
    return nc


_NC_CACHE = {}


def kernel(inputs_q, inputs_kv, bias, wq, wk, wv, wo):
    bf16 = ml_dtypes.bfloat16
    inputs_q = np.asarray(inputs_q)
    inputs_kv = np.asarray(inputs_kv)
    bias = np.asarray(bias)
    # fold the reference's 1/sqrt(D) query scaling into wq
    wq_s = (np.asarray(wq).reshape(E, H * D) / np.sqrt(D)).astype(bf16)
    wk_s = np.asarray(wk).reshape(E, H * D).astype(bf16)
    wv_s = np.asarray(wv).reshape(E, H * D).astype(bf16)
    wo_s = np.asarray(wo).reshape(H * D, E).astype(bf16)

    # host-side layout marshaling: embed-major activations, key-major exp(bias)
    xq_b = [np.ascontiguousarray(inputs_q[b].T).astype(bf16) for b in range(B)]
    xkv_b = [np.ascontiguousarray(inputs_kv[b].T).astype(bf16) for b in range(B)]
    expb_b = [np.exp(np.ascontiguousarray(bias[b, 0].T)).astype(bf16)
              for b in range(B)]

    in_maps = []
    for c in range(N_CORES):
        b, hg = c // 4, c % 4
        hs = slice(hg * HD, (hg + 1) * HD)
        in_maps.append({
            "xqT": xq_b[b],
            "xkvT": xkv_b[b],
            "expbT": expb_b[b],
            "wq": np.ascontiguousarray(wq_s[:, hs]),
            "wk": np.ascontiguousarray(wk_s[:, hs]),
            "wv": np.ascontiguousarray(wv_s[:, hs]),
            "wo": np.ascontiguousarray(wo_s[hs, :]),
        })

    if "nc" not in _NC_CACHE:
        _NC_CACHE["nc"] = build_program()
    nc = _NC_CACHE["nc"]

    res = run_bass_kernel_spmd(nc, in_maps, list(range(N_CORES)))
    outs = [np.asarray(r["out"], dtype=np.float32) for r in res.results]
    full = np.empty((B, T, E), dtype=np.float32)
    for b in range(B):
        full[b] = outs[4 * b] + outs[4 * b + 1] + outs[4 * b + 2] + outs[4 * b + 3]
    return full


# revision 10
# speedup vs baseline: 1.0148x; 1.0148x over previous
"""Multi-head dot-product attention (B=2, Q=K=2048, EMB=2048, H=32, D=64) on 8 TRN2 cores.

Sharding: data parallel over batch (2) x tensor parallel over heads (4 groups of 8).
Core c handles batch c//4, heads 8*(c%4) .. 8*(c%4)+8. Each core computes a partial
output [2048, 2048] (its heads' contribution through wo) in bf16; host sums the 4
head-group partials per batch in f32.

v5: the scalar engine's exp stream (~294us over 256 [128,1024] tiles) is the
attention pacer; the PE (~437us of bf16 matmul streaming) must deliver score
tiles at the scalar's rate *steadily* while soaking its own slack with finely-
grained independent work. Structure:
  - attention processes one head per (pair, hh) pass: per kc the PE owes only
    0.85us (scores+ctx) against the scalar's 1.15us, and the scores pipeline
    runs 2 kc ahead (3 psum slots) with ctx lagging 1 kc, so the
    scores->exp->mul->ctx round trip never blocks the in-order PE queue
  - independent PE work (q-projections for later chunks, out-projection of
    finished rows) is emitted in <=1.7us slices, 4 slots per pass, so the
    scalar never starves behind a long burst
  - K and V projections fill the pipe up front (scores consume all of kT/v
    within the first pass); exp(bias^T) is precomputed on the host
"""

import numpy as np
import ml_dtypes
from contextlib import ExitStack

import concourse.bass as bass
from concourse import bacc
import concourse.mybir as mybir
import concourse.tile as tile
from concourse.bass_utils import run_bass_kernel_spmd

BF16 = mybir.dt.bfloat16
F32 = mybir.dt.float32
AF = mybir.ActivationFunctionType

B, T, E = 2, 2048, 2048          # batch, tokens (Q=K), embed
H, D = 32, 64                     # total heads, head dim
NH = 8                            # heads per core
HD = NH * D                       # 512, per-core head-dim total
EC = E // 128                     # 16 contraction chunks
KC = T // 128                     # 16 key chunks
QCH = 1024                        # attention q-chunk (psum + exp tile width)
NQC = T // QCH                    # 2
N_CORES = 8


def build_program():
    nc = bacc.Bacc("TRN2", target_bir_lowering=False, debug=False,
                   num_devices=N_CORES)

    xqT = nc.dram_tensor("xqT", [E, T], BF16, kind="ExternalInput").ap()
    xkvT = nc.dram_tensor("xkvT", [E, T], BF16, kind="ExternalInput").ap()
    expbT = nc.dram_tensor("expbT", [T, T], BF16, kind="ExternalInput").ap()
    wq = nc.dram_tensor("wq", [E, HD], BF16, kind="ExternalInput").ap()
    wk = nc.dram_tensor("wk", [E, HD], BF16, kind="ExternalInput").ap()
    wv = nc.dram_tensor("wv", [E, HD], BF16, kind="ExternalInput").ap()
    wo = nc.dram_tensor("wo", [HD, E], BF16, kind="ExternalInput").ap()
    out = nc.dram_tensor("out", [T, E], BF16, kind="ExternalOutput").ap()

    with tile.TileContext(nc) as tc, ExitStack() as ctx:
        persist = ctx.enter_context(tc.tile_pool(name="persist", bufs=1))
        qT_sb = persist.tile([128, HD // 128, T], BF16)   # q^T[hd, t]
        kT_sb = persist.tile([128, HD // 128, T], BF16)
        v_sb = persist.tile([128, KC, NH, D + 1], BF16)   # v[k, h, d] + ones
        ctxT_sb = persist.tile([128, HD // 128, T], BF16)
        nc.vector.memset(v_sb[:, :, :, D:D + 1], 1.0)

        # wk's slot is reused for wo later (same tag)
        wpool = ctx.enter_context(tc.tile_pool(name="wpool", bufs=1))
        wk_sb = wpool.tile([128, EC, HD], BF16, tag="wkwo", name="wk_sb")
        wq_sb = wpool.tile([128, EC, HD], BF16, tag="wq", name="wq_sb")
        wv_sb = wpool.tile([128, EC, HD], BF16, tag="wv", name="wv_sb")

        expbp = ctx.enter_context(tc.tile_pool(name="expbp", bufs=1))
        xsp = ctx.enter_context(tc.tile_pool(name="xsp", bufs=2))
        # psum: 3 x s[128,1024] (6 banks) + 1 x ctx[65,1024] (2 banks)
        psp = ctx.enter_context(tc.tile_pool(name="psp", bufs=3, space="PSUM"))
        atp = ctx.enter_context(tc.tile_pool(name="atp", bufs=3))
        a2p = ctx.enter_context(tc.tile_pool(name="a2p", bufs=4))
        nrm = ctx.enter_context(tc.tile_pool(name="nrm", bufs=2))
        nrmd = ctx.enter_context(tc.tile_pool(name="nrmd", bufs=2, space="DRAM"))

        # ---------------- emission helpers ----------------
        def dma_chunked(dst, src_re, nchunk):
            step = EC // nchunk
            for i in range(nchunk):
                nc.gpsimd.dma_start(out=dst[:, i * step:(i + 1) * step, :],
                                    in_=src_re[:, i * step:(i + 1) * step, :])

        def x_stream(src, tc4, nchunk=1):
            """Stream activation chunk [128, EC, 512] for token cols tc4*512.."""
            xs = xsp.tile([128, EC, 512], BF16, name="xs", tag="xs")
            step = EC // nchunk
            for i in range(nchunk):
                nc.sync.dma_start(
                    out=xs[:, i * step:(i + 1) * step, :],
                    in_=bass.AP(tensor=src.tensor,
                                offset=src.offset + tc4 * 512 + i * step * 128 * T,
                                ap=[[T, 128], [128 * T, step], [1, 512]]))
            return xs

        def proj_mms(ps, w_sb, x_sb, hdc, e0, e1):
            for ec in range(e0, e1):
                nc.tensor.matmul(ps[:, 0:512],
                                 lhsT=w_sb[:, ec, hdc * 128:(hdc + 1) * 128],
                                 rhs=x_sb[:, ec, :],
                                 start=(ec == 0), stop=(ec == EC - 1))

        def proj_tile(w_sb, x_sb, dst, hdc, tc4):
            ps = psp.tile([128, QCH], F32, tag="s", name="ps")
            proj_mms(ps, w_sb, x_sb, hdc, 0, EC)
            nc.vector.tensor_copy(dst[:, hdc, tc4 * 512:(tc4 + 1) * 512],
                                  ps[:, 0:512])

        def v_tile(x_sb, tc4, sub):
            kc = tc4 * 4 + sub
            ps = psp.tile([128, QCH], F32, tag="s", name="vps")
            for ec in range(EC):
                nc.tensor.matmul(ps[:, 0:512],
                                 lhsT=x_sb[:, ec, sub * 128:(sub + 1) * 128],
                                 rhs=wv_sb[:, ec, :],
                                 start=(ec == 0), stop=(ec == EC - 1))
            nc.vector.tensor_copy(
                v_sb[:, kc, :, 0:D],
                ps[:, 0:512].rearrange("p (h d) -> p h d", h=NH))

        def expb_tile(kc, qc):
            eb = expbp.tile([128, QCH], BF16, tag=f"e{kc}", name=f"eb{kc}")
            nc.gpsimd.dma_start(
                out=eb[:],
                in_=expbT[kc * 128:(kc + 1) * 128, qc * QCH:(qc + 1) * QCH])
            return eb

        wo_ref = [None]

        def out_tile(i, copy_eng="v"):
            """Out-projection psum tile i (of 64): tc16 = i//4, e-quarter = i%4."""
            tc16, eq = i // 4, i % 4
            po = psp.tile([128, 512], F32, tag="s", name="po")
            for hdc in range(HD // 128):
                nc.tensor.matmul(
                    po[:],
                    lhsT=ctxT_sb[:, hdc, tc16 * 128:(tc16 + 1) * 128],
                    rhs=wo_ref[0][:, hdc, eq * 512:(eq + 1) * 512],
                    start=(hdc == 0), stop=(hdc == HD // 128 - 1))
            ot = nrm.tile([128, 512], BF16, tag="ostage", name="ot")
            if copy_eng == "v":
                nc.vector.tensor_copy(ot[:], po[:])
            else:
                nc.scalar.activation(ot[:], po[:], AF.Copy)
            nc.sync.dma_start(
                out=out[tc16 * 128:(tc16 + 1) * 128, eq * 512:(eq + 1) * 512],
                in_=ot[:])

        def emit_score(pair, hh, qc, kc):
            pr = slice(hh * D, (hh + 1) * D)
            s = psp.tile([128, QCH], F32, tag="s", name="s")
            for half in range(QCH // 512):
                q0 = qc * QCH + half * 512
                nc.tensor.matmul(
                    s[:, half * 512:(half + 1) * 512],
                    lhsT=kT_sb[pr, pair, kc * 128:(kc + 1) * 128],
                    rhs=qT_sb[pr, pair, q0:q0 + 512],
                    start=True, stop=True)
            return s

        def emit_expmul(s_tile, eb):
            at = atp.tile([128, QCH], BF16, tag="at", name="at")
            nc.scalar.activation(at[:], s_tile[:], AF.Exp)
            a2 = a2p.tile([128, QCH], BF16, tag="a2", name="a2")
            nc.vector.tensor_mul(a2[:], at[:], eb[:])
            return a2

        def emit_ctx(ctx_t, pair, hh, kc, a2):
            h = pair * 2 + hh
            for half in range(QCH // 512):
                nc.tensor.matmul(
                    ctx_t[:, half * 512:(half + 1) * 512],
                    lhsT=v_sb[:, kc, h, :],
                    rhs=a2[:, half * 512:(half + 1) * 512],
                    start=(kc == 0), stop=(kc == KC - 1))

        def norm_hh(ctx_t, pair, hh, qc):
            ctxf = nrm.tile([D + 1, QCH], BF16, tag="ctxf", name="ctxf")
            nc.vector.tensor_copy(ctxf[:], ctx_t[:])  # frees the psum slot
            srow = nrm.tile([128, QCH // 128], F32, tag="srow", name="srow")
            nc.gpsimd.dma_start(out=srow[:], in_=ctxf[D:D + 1, :])
            rec = nrm.tile([128, QCH // 128], F32, tag="rec", name="rec")
            nc.vector.reciprocal_approx_fast(out=rec[:], in_=srow[:])
            rec_d = nrmd.tile([QCH], F32, tag="recd", name="recd")
            nc.sync.dma_start(
                out=rec_d[:].rearrange("(p j) -> p j", p=128), in_=rec[:])
            recb = nrm.tile([D, QCH], BF16, tag="recb", name="recb")
            rd = rec_d[:]
            bcast = bass.AP(tensor=rd.tensor, offset=rd.offset,
                            ap=[[0, D]] + list(rd.ap))
            nc.gpsimd.dma_start(out=recb[:], in_=bcast)  # casts f32->bf16
            if hh == 0:
                nc.vector.tensor_mul(
                    ctxT_sb[0:D, pair, qc * QCH:(qc + 1) * QCH],
                    ctxf[0:D, :], recb[:])
            else:
                stage = nrm.tile([D, QCH], BF16, tag="cstage", name="stg")
                nc.vector.tensor_mul(stage[:], ctxf[0:D, :], recb[:])
                nc.sync.dma_start(
                    out=ctxT_sb[D:2 * D, pair, qc * QCH:(qc + 1) * QCH],
                    in_=stage[:])

        # ---------------- fill phase ----------------
        dma_chunked(wk_sb, wk.rearrange("(ec p) n -> p ec n", p=128), 4)
        dma_chunked(wv_sb, wv.rearrange("(ec p) n -> p ec n", p=128), 2)
        dma_chunked(wq_sb, wq.rearrange("(ec p) n -> p ec n", p=128), 2)
        expb_tiles = {kc: expb_tile(kc, 0) for kc in range(KC)}

        xq_fill = []
        for tc4 in range(4):
            xs = x_stream(xkvT, tc4, nchunk=(4 if tc4 == 0 else 2))
            for hdc in range(4):
                proj_tile(wk_sb, xs, kT_sb, hdc, tc4)
            for sub in range(4):
                v_tile(xs, tc4, sub)
        for tc4 in range(2):
            xs = x_stream(xqT, tc4)
            xq_fill.append(xs)
        # q-projection for (pair0, qc0) only; the rest interleaves into attention
        proj_tile(wq_sb, xq_fill[0], qT_sb, 0, 0)
        proj_tile(wq_sb, xq_fill[1], qT_sb, 0, 1)

        # wo reuses wk's slot; its DMA waits on the last wk reader
        wo_sb = wpool.tile([128, HD // 128, E], BF16, tag="wkwo", name="wo_sb")
        nc.gpsimd.dma_start(out=wo_sb[:],
                            in_=wo.rearrange("(c p) n -> p c n", p=128))
        wo_ref[0] = wo_sb

        # ------------- interleave supply (fine-grained thunks) -------------
        xq_late = {}

        def make_proj_halves(src_tiles, hdc, tc4):
            """A q-projection tile as two 8-matmul thunks."""
            state = {}

            def first():
                state["ps"] = psp.tile([128, QCH], F32, tag="s", name="ps")
                proj_mms(state["ps"], wq_sb, src_tiles[tc4], hdc, 0, EC // 2)

            def second():
                proj_mms(state["ps"], wq_sb, src_tiles[tc4], hdc, EC // 2, EC)
                nc.vector.tensor_copy(
                    qT_sb[:, hdc, tc4 * 512:(tc4 + 1) * 512],
                    state["ps"][:, 0:512])
            return [first, second]

        def make_stream_thunk(tc4):
            def go():
                xq_late[tc4] = x_stream(xqT, tc4)
            return [go]

        supply = {0: [], 1: []}
        xq_fill_d = {0: xq_fill[0], 1: xq_fill[1]}
        for p in (1, 2, 3):                      # q-proj qc0 for pairs 1-3
            for tc4 in (0, 1):
                supply[0] += make_proj_halves(xq_fill_d, p, tc4)
        supply[0] += make_stream_thunk(2) + make_stream_thunk(3)
        for p in (0, 1, 2, 3):                   # q-proj qc1
            for tc4 in (2, 3):
                supply[0] += make_proj_halves(xq_late, p, tc4)
        supply[1] += [lambda i=i: out_tile(i) for i in range(32)]

        def slot_budget(qc, win, kc):
            """Interleave thunks to emit at (window, kc): always at most ONE
            thunk per slot (a slot is <=1.7us of PE work) so the scalar's exp
            stream is never starved behind a long PE burst. supA (q-proj qc0 +
            xq streams, 14 thunks) drains in windows 0-1; supB (q-proj qc1,
            needs the late xq streams landed) starts from window 3."""
            if qc == 1:
                return 1 if kc % 4 == 2 else 0
            if win <= 1:
                return 1 if kc % 2 == 0 else 0
            if win == 2:
                return 0
            return 1 if kc % 4 == 2 else 0

        # ---------------- attention ----------------
        for qc in range(NQC):
            sup = supply[qc]
            si = 0
            for pair in range(NH // 2):
                for hh in range(2):
                    win = pair * 2 + hh
                    ctx_t = psp.tile([D + 1, QCH], F32, tag="ctx", bufs=1,
                                     name="ctx")
                    s_tiles = {0: emit_score(pair, hh, qc, 0),
                               1: emit_score(pair, hh, qc, 1)}
                    a2_hist = {}
                    for kc in range(KC):
                        a2_hist[kc] = emit_expmul(s_tiles.pop(kc),
                                                  expb_tiles[kc])
                        if qc == 0 and pair == 3 and hh == 1:
                            expb_tiles[kc] = expb_tile(kc, 1)
                        if kc + 2 < KC:
                            s_tiles[kc + 2] = emit_score(pair, hh, qc, kc + 2)
                        for _ in range(slot_budget(qc, win, kc)):
                            if si < len(sup):
                                sup[si]()
                                si += 1
                        if kc >= 1:
                            emit_ctx(ctx_t, pair, hh, kc - 1,
                                     a2_hist.pop(kc - 1))
                    emit_ctx(ctx_t, pair, hh, KC - 1, a2_hist.pop(KC - 1))
                    norm_hh(ctx_t, pair, hh, qc)
            # drain any unused supply at qc end
            while si < len(sup):
                sup[si]()
                si += 1

        # ---------------- tail: out-proj for qc1 rows ----------------
        for i in range(32, 64):
            out_tile(i, copy_eng=("s" if i % 2 else "v"))

    nc.compile()
    return nc


_NC_CACHE = {}


def kernel(inputs_q, inputs_kv, bias, wq, wk, wv, wo):
    bf16 = ml_dtypes.bfloat16
    inputs_q = np.asarray(inputs_q)
    inputs_kv = np.asarray(inputs_kv)
    bias = np.asarray(bias)
    # fold the reference's 1/sqrt(D) query scaling into wq
    wq_s = (np.asarray(wq).reshape(E, H * D) / np.sqrt(D)).astype(bf16)
    wk_s = np.asarray(wk).reshape(E, H * D).astype(bf16)
    wv_s = np.asarray(wv).reshape(E, H * D).astype(bf16)
    wo_s = np.asarray(wo).reshape(H * D, E).astype(bf16)

    # host-side layout marshaling: embed-major activations, key-major exp(bias)
    xq_b = [np.ascontiguousarray(inputs_q[b].T).astype(bf16) for b in range(B)]
    xkv_b = [np.ascontiguousarray(inputs_kv[b].T).astype(bf16) for b in range(B)]
    expb_b = [np.exp(np.ascontiguousarray(bias[b, 0].T)).astype(bf16)
              for b in range(B)]

    in_maps = []
    for c in range(N_CORES):
        b, hg = c // 4, c % 4
        hs = slice(hg * HD, (hg + 1) * HD)
        in_maps.append({
            "xqT": xq_b[b],
            "xkvT": xkv_b[b],
            "expbT": expb_b[b],
            "wq": np.ascontiguousarray(wq_s[:, hs]),
            "wk": np.ascontiguousarray(wk_s[:, hs]),
            "wv": np.ascontiguousarray(wv_s[:, hs]),
            "wo": np.ascontiguousarray(wo_s[hs, :]),
        })

    if "nc" not in _NC_CACHE:
        _NC_CACHE["nc"] = build_program()
    nc = _NC_CACHE["nc"]

    res = run_bass_kernel_spmd(nc, in_maps, list(range(N_CORES)))
    outs = [np.asarray(r["out"], dtype=np.float32) for r in res.results]
    full = np.empty((B, T, E), dtype=np.float32)
    for b in range(B):
        full[b] = outs[4 * b] + outs[4 * b + 1] + outs[4 * b + 2] + outs[4 * b + 3]
    return full


# revision 11
# speedup vs baseline: 1.0509x; 1.0356x over previous
"""Multi-head dot-product attention (B=2, Q=K=2048, EMB=2048, H=32, D=64) on 8 TRN2 cores.

Sharding: data parallel over batch (2) x tensor parallel over heads (4 groups of 8).
Core c handles batch c//4, heads 8*(c%4) .. 8*(c%4)+8. Each core computes a partial
output [2048, 2048] (its heads' contribution through wo) in bf16; host sums the 4
head-group partials per batch in f32.

v5: the scalar engine's exp stream (~294us over 256 [128,1024] tiles) is the
attention pacer; the PE (~437us of bf16 matmul streaming) must deliver score
tiles at the scalar's rate *steadily* while soaking its own slack with finely-
grained independent work. Structure:
  - attention processes one head per (pair, hh) pass: per kc the PE owes only
    0.85us (scores+ctx) against the scalar's 1.15us, and the scores pipeline
    runs 2 kc ahead (3 psum slots) with ctx lagging 1 kc, so the
    scores->exp->mul->ctx round trip never blocks the in-order PE queue
  - independent PE work (q-projections for later chunks, out-projection of
    finished rows) is emitted in <=1.7us slices, 4 slots per pass, so the
    scalar never starves behind a long burst
  - K and V projections fill the pipe up front (scores consume all of kT/v
    within the first pass); exp(bias^T) is precomputed on the host
"""

import numpy as np
import ml_dtypes
from contextlib import ExitStack

import concourse.bass as bass
from concourse import bacc
import concourse.mybir as mybir
import concourse.tile as tile
from concourse.bass_utils import run_bass_kernel_spmd

BF16 = mybir.dt.bfloat16
F32 = mybir.dt.float32
AF = mybir.ActivationFunctionType

B, T, E = 2, 2048, 2048          # batch, tokens (Q=K), embed
H, D = 32, 64                     # total heads, head dim
NH = 8                            # heads per core
HD = NH * D                       # 512, per-core head-dim total
EC = E // 128                     # 16 contraction chunks
KC = T // 128                     # 16 key chunks
QCH = 1024                        # attention q-chunk (psum + exp tile width)
NQC = T // QCH                    # 2
N_CORES = 8


def build_program():
    nc = bacc.Bacc("TRN2", target_bir_lowering=False, debug=False,
                   num_devices=N_CORES)

    xqT = nc.dram_tensor("xqT", [E, T], BF16, kind="ExternalInput").ap()
    xkvT = nc.dram_tensor("xkvT", [E, T], BF16, kind="ExternalInput").ap()
    expbT = nc.dram_tensor("expbT", [T, T], BF16, kind="ExternalInput").ap()
    wq = nc.dram_tensor("wq", [E, HD], BF16, kind="ExternalInput").ap()
    wk = nc.dram_tensor("wk", [E, HD], BF16, kind="ExternalInput").ap()
    wv = nc.dram_tensor("wv", [E, HD], BF16, kind="ExternalInput").ap()
    wo = nc.dram_tensor("wo", [HD, E], BF16, kind="ExternalInput").ap()
    out = nc.dram_tensor("out", [T, E], BF16, kind="ExternalOutput").ap()

    with tile.TileContext(nc) as tc, ExitStack() as ctx:
        persist = ctx.enter_context(tc.tile_pool(name="persist", bufs=1))
        qT_sb = persist.tile([128, HD // 128, T], BF16)   # q^T[hd, t]
        kT_sb = persist.tile([128, HD // 128, T], BF16)
        v_sb = persist.tile([128, KC, NH, D + 1], BF16)   # v[k, h, d] + ones
        ctxT_sb = persist.tile([128, HD // 128, T], BF16)
        nc.vector.memset(v_sb[:, :, :, D:D + 1], 1.0)

        # wk's slot is reused for wo later (same tag)
        wpool = ctx.enter_context(tc.tile_pool(name="wpool", bufs=1))
        wk_sb = wpool.tile([128, EC, HD], BF16, tag="wkwo", name="wk_sb")
        wq_sb = wpool.tile([128, EC, HD], BF16, tag="wq", name="wq_sb")
        wv_sb = wpool.tile([128, EC, HD], BF16, tag="wv", name="wv_sb")

        expbp = ctx.enter_context(tc.tile_pool(name="expbp", bufs=1))
        xsp = ctx.enter_context(tc.tile_pool(name="xsp", bufs=2))
        # psum: 3 x s[128,1024] (6 banks) + 1 x ctx[65,1024] (2 banks)
        psp = ctx.enter_context(tc.tile_pool(name="psp", bufs=3, space="PSUM"))
        atp = ctx.enter_context(tc.tile_pool(name="atp", bufs=3))
        a2p = ctx.enter_context(tc.tile_pool(name="a2p", bufs=4))
        nrm = ctx.enter_context(tc.tile_pool(name="nrm", bufs=2))
        nrmd = ctx.enter_context(tc.tile_pool(name="nrmd", bufs=2, space="DRAM"))

        # ---------------- emission helpers ----------------
        def dma_chunked(dst, src_re, nchunk):
            step = EC // nchunk
            for i in range(nchunk):
                nc.gpsimd.dma_start(out=dst[:, i * step:(i + 1) * step, :],
                                    in_=src_re[:, i * step:(i + 1) * step, :])

        def x_stream(src, tc4, nchunk=1):
            """Stream activation chunk [128, EC, 512] for token cols tc4*512.."""
            xs = xsp.tile([128, EC, 512], BF16, name="xs", tag="xs")
            step = EC // nchunk
            for i in range(nchunk):
                nc.sync.dma_start(
                    out=xs[:, i * step:(i + 1) * step, :],
                    in_=bass.AP(tensor=src.tensor,
                                offset=src.offset + tc4 * 512 + i * step * 128 * T,
                                ap=[[T, 128], [128 * T, step], [1, 512]]))
            return xs

        def proj_mms(ps, w_sb, x_sb, hdc, e0, e1):
            for ec in range(e0, e1):
                nc.tensor.matmul(ps[:, 0:512],
                                 lhsT=w_sb[:, ec, hdc * 128:(hdc + 1) * 128],
                                 rhs=x_sb[:, ec, :],
                                 start=(ec == 0), stop=(ec == EC - 1))

        def proj_tile(w_sb, x_sb, dst, hdc, tc4):
            ps = psp.tile([128, QCH], F32, tag="s", name="ps")
            proj_mms(ps, w_sb, x_sb, hdc, 0, EC)
            nc.vector.tensor_copy(dst[:, hdc, tc4 * 512:(tc4 + 1) * 512],
                                  ps[:, 0:512])

        def v_tile(x_sb, tc4, sub):
            kc = tc4 * 4 + sub
            ps = psp.tile([128, QCH], F32, tag="s", name="vps")
            for ec in range(EC):
                nc.tensor.matmul(ps[:, 0:512],
                                 lhsT=x_sb[:, ec, sub * 128:(sub + 1) * 128],
                                 rhs=wv_sb[:, ec, :],
                                 start=(ec == 0), stop=(ec == EC - 1))
            nc.vector.tensor_copy(
                v_sb[:, kc, :, 0:D],
                ps[:, 0:512].rearrange("p (h d) -> p h d", h=NH))

        def expb_tile(kc, qc):
            eb = expbp.tile([128, QCH], BF16, tag=f"e{kc}", name=f"eb{kc}")
            nc.gpsimd.dma_start(
                out=eb[:],
                in_=expbT[kc * 128:(kc + 1) * 128, qc * QCH:(qc + 1) * QCH])
            return eb

        wo_ref = [None]

        def out_tile(i, copy_eng="v"):
            """Out-projection psum tile i (of 64): tc16 = i//4, e-quarter = i%4."""
            tc16, eq = i // 4, i % 4
            po = psp.tile([128, 512], F32, tag="s", name="po")
            for hdc in range(HD // 128):
                nc.tensor.matmul(
                    po[:],
                    lhsT=ctxT_sb[:, hdc, tc16 * 128:(tc16 + 1) * 128],
                    rhs=wo_ref[0][:, hdc, eq * 512:(eq + 1) * 512],
                    start=(hdc == 0), stop=(hdc == HD // 128 - 1))
            ot = nrm.tile([128, 512], BF16, tag="ostage", name="ot")
            if copy_eng == "v":
                nc.vector.tensor_copy(ot[:], po[:])
            else:
                nc.scalar.activation(ot[:], po[:], AF.Copy)
            nc.sync.dma_start(
                out=out[tc16 * 128:(tc16 + 1) * 128, eq * 512:(eq + 1) * 512],
                in_=ot[:])

        def emit_score(pair, hh, qc, kc):
            pr = slice(hh * D, (hh + 1) * D)
            s = psp.tile([128, QCH], F32, tag="s", name="s")
            for half in range(QCH // 512):
                q0 = qc * QCH + half * 512
                nc.tensor.matmul(
                    s[:, half * 512:(half + 1) * 512],
                    lhsT=kT_sb[pr, pair, kc * 128:(kc + 1) * 128],
                    rhs=qT_sb[pr, pair, q0:q0 + 512],
                    start=True, stop=True)
            return s

        def emit_expmul(s_tile, eb):
            at = atp.tile([128, QCH], BF16, tag="at", name="at")
            nc.scalar.activation(at[:], s_tile[:], AF.Exp)
            a2 = a2p.tile([128, QCH], BF16, tag="a2", name="a2")
            nc.vector.tensor_mul(a2[:], at[:], eb[:])
            return a2

        def emit_ctx(ctx_t, pair, hh, kc, a2):
            h = pair * 2 + hh
            for half in range(QCH // 512):
                nc.tensor.matmul(
                    ctx_t[:, half * 512:(half + 1) * 512],
                    lhsT=v_sb[:, kc, h, :],
                    rhs=a2[:, half * 512:(half + 1) * 512],
                    start=(kc == 0), stop=(kc == KC - 1))

        def norm_hh(ctx_t, pair, hh, qc):
            ctxf = nrm.tile([D + 1, QCH], BF16, tag="ctxf", name="ctxf")
            nc.vector.tensor_copy(ctxf[:], ctx_t[:])  # frees the psum slot
            srow = nrm.tile([128, QCH // 128], F32, tag="srow", name="srow")
            nc.gpsimd.dma_start(out=srow[:], in_=ctxf[D:D + 1, :])
            rec = nrm.tile([128, QCH // 128], F32, tag="rec", name="rec")
            nc.vector.reciprocal_approx_fast(out=rec[:], in_=srow[:])
            rec_d = nrmd.tile([QCH], F32, tag="recd", name="recd")
            nc.sync.dma_start(
                out=rec_d[:].rearrange("(p j) -> p j", p=128), in_=rec[:])
            recb = nrm.tile([D, QCH], BF16, tag="recb", name="recb")
            rd = rec_d[:]
            bcast = bass.AP(tensor=rd.tensor, offset=rd.offset,
                            ap=[[0, D]] + list(rd.ap))
            nc.gpsimd.dma_start(out=recb[:], in_=bcast)  # casts f32->bf16
            if hh == 0:
                nc.vector.tensor_mul(
                    ctxT_sb[0:D, pair, qc * QCH:(qc + 1) * QCH],
                    ctxf[0:D, :], recb[:])
            else:
                stage = nrm.tile([D, QCH], BF16, tag="cstage", name="stg")
                nc.vector.tensor_mul(stage[:], ctxf[0:D, :], recb[:])
                nc.sync.dma_start(
                    out=ctxT_sb[D:2 * D, pair, qc * QCH:(qc + 1) * QCH],
                    in_=stage[:])

        # ---------------- fill phase ----------------
        dma_chunked(wk_sb, wk.rearrange("(ec p) n -> p ec n", p=128), 4)
        dma_chunked(wv_sb, wv.rearrange("(ec p) n -> p ec n", p=128), 2)
        dma_chunked(wq_sb, wq.rearrange("(ec p) n -> p ec n", p=128), 2)
        expb_tiles = {kc: expb_tile(kc, 0) for kc in range(KC)}

        xq_fill = []
        for tc4 in range(4):
            xs = x_stream(xkvT, tc4, nchunk=(4 if tc4 == 0 else 2))
            for hdc in range(4):
                proj_tile(wk_sb, xs, kT_sb, hdc, tc4)
            for sub in range(4):
                v_tile(xs, tc4, sub)
        for tc4 in range(2):
            xs = x_stream(xqT, tc4)
            xq_fill.append(xs)
        # q-projection for all of qc0 (scores consume it pair by pair)
        for p in range(4):
            proj_tile(wq_sb, xq_fill[0], qT_sb, p, 0)
            proj_tile(wq_sb, xq_fill[1], qT_sb, p, 1)

        # wo reuses wk's slot; its DMA waits on the last wk reader
        wo_sb = wpool.tile([128, HD // 128, E], BF16, tag="wkwo", name="wo_sb")
        nc.gpsimd.dma_start(out=wo_sb[:],
                            in_=wo.rearrange("(c p) n -> p c n", p=128))
        wo_ref[0] = wo_sb

        # ------------- interleave supply (fine-grained thunks) -------------
        xq_late = {}

        def make_proj_halves(src_tiles, hdc, tc4):
            """A q-projection tile as two 8-matmul thunks."""
            state = {}

            def first():
                state["ps"] = psp.tile([128, QCH], F32, tag="s", name="ps")
                proj_mms(state["ps"], wq_sb, src_tiles[tc4], hdc, 0, EC // 2)

            def second():
                proj_mms(state["ps"], wq_sb, src_tiles[tc4], hdc, EC // 2, EC)
                nc.vector.tensor_copy(
                    qT_sb[:, hdc, tc4 * 512:(tc4 + 1) * 512],
                    state["ps"][:, 0:512])
            return [first, second]

        def make_stream_thunk(tc4):
            def go():
                xq_late[tc4] = x_stream(xqT, tc4)
            return [go]

        supply = {0: [], 1: []}
        supply[0] += make_stream_thunk(2) + make_stream_thunk(3)
        for p in (0, 1, 2, 3):                   # q-proj qc1
            for tc4 in (2, 3):
                supply[0] += make_proj_halves(xq_late, p, tc4)
        supply[1] += [lambda i=i: out_tile(i) for i in range(32)]

        def slot_budget(qc, win, kc):
            """Interleave thunks to emit at (window, kc): always at most ONE
            thunk per slot (a slot is <=1.7us of PE work) so the scalar's exp
            stream is never starved behind a long PE burst. supA (q-proj qc0 +
            xq streams, 14 thunks) drains in windows 0-1; supB (q-proj qc1,
            needs the late xq streams landed) starts from window 3."""
            if qc == 1:
                return 1 if kc % 4 == 2 else 0
            if win == 0:
                return 1 if kc in (6, 12) else 0   # the two stream thunks
            if win == 1:
                return 0                           # let xq_late DMAs land
            return 1 if kc in (4, 12) else 0       # q-proj qc1 halves

        # ---------------- attention ----------------
        for qc in range(NQC):
            sup = supply[qc]
            si = 0
            for pair in range(NH // 2):
                for hh in range(2):
                    win = pair * 2 + hh
                    ctx_t = psp.tile([D + 1, QCH], F32, tag="ctx", bufs=1,
                                     name="ctx")
                    s_tiles = {0: emit_score(pair, hh, qc, 0),
                               1: emit_score(pair, hh, qc, 1)}
                    a2_hist = {}
                    for kc in range(KC):
                        a2_hist[kc] = emit_expmul(s_tiles.pop(kc),
                                                  expb_tiles[kc])
                        if qc == 0 and pair == 3 and hh == 1:
                            expb_tiles[kc] = expb_tile(kc, 1)
                        if kc + 2 < KC:
                            s_tiles[kc + 2] = emit_score(pair, hh, qc, kc + 2)
                        for _ in range(slot_budget(qc, win, kc)):
                            if si < len(sup):
                                sup[si]()
                                si += 1
                        if kc >= 1:
                            emit_ctx(ctx_t, pair, hh, kc - 1,
                                     a2_hist.pop(kc - 1))
                    emit_ctx(ctx_t, pair, hh, KC - 1, a2_hist.pop(KC - 1))
                    norm_hh(ctx_t, pair, hh, qc)
            # drain any unused supply at qc end
            while si < len(sup):
                sup[si]()
                si += 1

        # ---------------- tail: out-proj for qc1 rows ----------------
        for i in range(32, 64):
            out_tile(i, copy_eng=("s" if i % 2 else "v"))

    nc.compile()
    return nc


_NC_CACHE = {}


def kernel(inputs_q, inputs_kv, bias, wq, wk, wv, wo):
    bf16 = ml_dtypes.bfloat16
    inputs_q = np.asarray(inputs_q)
    inputs_kv = np.asarray(inputs_kv)
    bias = np.asarray(bias)
    # fold the reference's 1/sqrt(D) query scaling into wq
    wq_s = (np.asarray(wq).reshape(E, H * D) / np.sqrt(D)).astype(bf16)
    wk_s = np.asarray(wk).reshape(E, H * D).astype(bf16)
    wv_s = np.asarray(wv).reshape(E, H * D).astype(bf16)
    wo_s = np.asarray(wo).reshape(H * D, E).astype(bf16)

    # host-side layout marshaling: embed-major activations, key-major exp(bias)
    xq_b = [np.ascontiguousarray(inputs_q[b].T).astype(bf16) for b in range(B)]
    xkv_b = [np.ascontiguousarray(inputs_kv[b].T).astype(bf16) for b in range(B)]
    expb_b = [np.exp(np.ascontiguousarray(bias[b, 0].T)).astype(bf16)
              for b in range(B)]

    in_maps = []
    for c in range(N_CORES):
        b, hg = c // 4, c % 4
        hs = slice(hg * HD, (hg + 1) * HD)
        in_maps.append({
            "xqT": xq_b[b],
            "xkvT": xkv_b[b],
            "expbT": expb_b[b],
            "wq": np.ascontiguousarray(wq_s[:, hs]),
            "wk": np.ascontiguousarray(wk_s[:, hs]),
            "wv": np.ascontiguousarray(wv_s[:, hs]),
            "wo": np.ascontiguousarray(wo_s[hs, :]),
        })

    if "nc" not in _NC_CACHE:
        _NC_CACHE["nc"] = build_program()
    nc = _NC_CACHE["nc"]

    res = run_bass_kernel_spmd(nc, in_maps, list(range(N_CORES)))
    outs = [np.asarray(r["out"], dtype=np.float32) for r in res.results]
    full = np.empty((B, T, E), dtype=np.float32)
    for b in range(B):
        full[b] = outs[4 * b] + outs[4 * b + 1] + outs[4 * b + 2] + outs[4 * b + 3]
    return full
